# revision 1
# baseline (speedup 1.0000x reference)
"""Trainium2 Bass kernel for nn_BakedAttentionHead.

Reference computation (per row b of query):
    s      = (q @ K^T) / sqrt(D)                      # (B, N)
    e'     = exp(s - max_n s)
    d      = 1 + sum_n e'
    recip  = 16-step sigmoid long-division approx of 1/d
    out    = (e' * recip) @ V

Kernel restructuring (algebraically equivalent, fp-wise ~1e-7 of reference):
    e      = exp(s)                 (raw; |s| <= ~6 so no overflow)
    em     = exp(-max_n s)
    d      = 1 + (sum_n e) * em
    out    = (e @ V) * (em * recip) per row

Sharding: data-parallel over the 8192 query rows -> 8 cores x 1024 rows,
keys/values replicated.  Matmuls run in float32r (full-rate fp32 PE mode).
mm1 computes scores^T ([n, m] orientation, 512 m per pair of output blocks)
so the exp'd tiles are directly the lhsT operand of mm2 with no transposes
of the big intermediate; only the tiny [128, 512] max/sum stat tensors go
through PE transposes for the cross-partition reduction.  The sigmoid
long-division scan runs between the mm2 compute and the output scale pass,
so neither the PE nor the PSUM-evacuating ACT queue ever waits on it.
"""

import numpy as np

B, D, N = 8192, 1024, 2048
NCORES = 8
M = B // NCORES            # 1024 query rows per core
NPAIR = 2                  # m "pairs" per core (one mm1 sweep each)
PW = M // NPAIR            # 512 m per pair = mm1 moving free dim
MT = PW // 128             # 4 output m-tiles of 128 rows per pair
NT = N // 128              # 16 n tiles
DT = D // 128              # 8 d (contraction) tiles
DO = 2                     # output dout chunks of 512
SCALE = 0.03125            # D ** -0.5
SIG_SCALE = 100.0
BITS = 16

_CACHE = {}


def _build(reps=1):
    import concourse.mybir as mybir
    import concourse.tile as tile
    from concourse import bacc
    from concourse.masks import make_identity
    from concourse.tile import add_dep_helper

    F32 = mybir.dt.float32
    F32R = mybir.dt.float32r
    AX = mybir.AxisListType
    OP = mybir.AluOpType
    AF = mybir.ActivationFunctionType

    nc = bacc.Bacc("TRN2", target_bir_lowering=False, debug=False,
                   num_devices=NCORES)
    qT_d = nc.declare_dram_parameter("qT", [D, M], F32R, isOutput=False)
    kT_d = nc.declare_dram_parameter("kT", [D, N], F32R, isOutput=False)
    v_d = nc.declare_dram_parameter("v", [N, D], F32R, isOutput=False)
    out_d = nc.declare_dram_parameter("out", [M, D], F32, isOutput=True)

    qT_ap = qT_d[:].rearrange("(dt p) m -> p dt m", p=128)
    kT_ap = kT_d[:].rearrange("(dt p) n -> p dt n", p=128)
    v_ap = v_d[:].rearrange("(nt p) do -> p nt do", p=128)

    with tile.TileContext(nc) as tc:
        with (
            tc.tile_pool(name="res", bufs=1) as res_pool,
            tc.tile_pool(name="e", bufs=NT) as e_pool,
            tc.tile_pool(name="acc", bufs=1) as acc_pool,
            tc.tile_pool(name="qt", bufs=1) as qt_pool,
            tc.tile_pool(name="stat", bufs=2) as stat_pool,
            tc.tile_pool(name="o", bufs=8) as out_pool,
            tc.tile_pool(name="ps1", bufs=3, space="PSUM") as ps1_pool,
            tc.tile_pool(name="ps2", bufs=5, space="PSUM") as ps2_pool,
        ):
            ident = res_pool.tile([128, 128], F32)
            make_identity(nc, ident[:])

            for rep in range(reps):
                # The SP HWDGE queue is FIFO and HBM bandwidth is one shared
                # pipe: emit loads in first-use order (kt + pair-0 qt gating
                # mm1 first, vt last -- it is only needed once mm2 starts).
                # qt streams per pair from a single slot: pair 1's load can
                # only start once pair 0's mm1 is done, which lands during
                # mm2(p0), well before mm1(p1) needs it.
                kt = res_pool.tile([128, DT, N], F32R)
                vt = res_pool.tile([128, NT, D], F32R)
                qts = []
                # fine-grained first loads: mm1's first matmuls need only the
                # first 128-n slice of kt plus qt0 dt-slices, so stage those
                # ahead of the bulk chunks to start the PE ~10 us earlier
                nc.sync.dma_start(out=kt[:, :, 0:128], in_=kT_ap[:, :, 0:128])
                qt0 = qt_pool.tile([128, DT, PW], F32R, name="qt0", tag="qt")
                for c in range(4):
                    nc.sync.dma_start(out=qt0[:, c * 2:(c + 1) * 2, :],
                                      in_=qT_ap[:, c * 2:(c + 1) * 2, 0:PW])
                qts.append(qt0)
                nc.sync.dma_start(out=kt[:, :, 128:512], in_=kT_ap[:, :, 128:512])
                for c in range(1, 4):
                    nc.sync.dma_start(out=kt[:, :, c * 512:(c + 1) * 512],
                                      in_=kT_ap[:, :, c * 512:(c + 1) * 512])
                for c in range(4):
                    nc.sync.dma_start(out=vt[:, c * 4:(c + 1) * 4, :],
                                      in_=v_ap[:, c * 4:(c + 1) * 4, :])
                qt1 = qt_pool.tile([128, DT, PW], F32R, name="qt1", tag="qt")
                nc.sync.dma_start(out=qt1[:], in_=qT_ap[:, :, PW:M])
                qts.append(qt1)

                def emit_mm1(p, st):
                    """scores^T for pair p: psum [128 n, 512 m] per n tile,
                    exp'd into f32r e tiles; elementwise max/sum accumulation
                    across n tiles; cross-partition stats via PE transpose."""
                    etiles = []
                    macc = acc_pool.tile([128, PW], F32, name=f"macc{p}",
                                         tag="macc")
                    sacc = acc_pool.tile([128, PW], F32, name=f"sacc{p}",
                                         tag="sacc")
                    for nt in range(NT):
                        ps = ps1_pool.tile([128, PW], F32, name=f"s{p}_{nt}",
                                           tag="ps1")
                        for dt in range(DT):
                            nc.tensor.matmul(
                                ps[:],
                                lhsT=kt[:, dt, nt * 128:(nt + 1) * 128],
                                rhs=qts[p][:, dt, :],
                                start=(dt == 0), stop=(dt == DT - 1),
                            )
                        e_nt = e_pool.tile([128, PW], F32R, name=f"e{p}_{nt}",
                                           tag="e")
                        nc.scalar.activation(e_nt[:], ps[:], AF.Exp, scale=SCALE)
                        if nt == 0:
                            nc.vector.tensor_copy(macc[:], ps[:])
                            nc.vector.tensor_copy(sacc[:], e_nt[:].bitcast(F32))
                        else:
                            nc.vector.tensor_tensor(
                                out=macc[:], in0=ps[:], in1=macc[:], op=OP.max)
                            nc.vector.tensor_tensor(
                                out=sacc[:], in0=e_nt[:].bitcast(F32),
                                in1=sacc[:], op=OP.add)
                        etiles.append(e_nt)
                    return etiles, macc, sacc

                def emit_stats(p, st, macc, sacc):
                    mx = stat_pool.tile([128, MT], F32, name=f"mx{p}", tag="mx")
                    sm = stat_pool.tile([128, MT], F32, name=f"sm{p}", tag="sm")
                    for c in range(MT):
                        pt = ps1_pool.tile([128, 128], F32, name=f"tm{p}_{c}",
                                           tag="ps1")
                        nc.tensor.transpose(
                            pt[:], macc[:, c * 128:(c + 1) * 128], ident[:])
                        nc.vector.tensor_reduce(
                            mx[:, c:c + 1], pt[:], axis=AX.X, op=OP.max)
                        pt2 = ps1_pool.tile([128, 128], F32, name=f"ts{p}_{c}",
                                            tag="ps1")
                        nc.tensor.transpose(
                            pt2[:], sacc[:, c * 128:(c + 1) * 128], ident[:])
                        nc.vector.tensor_reduce(
                            sm[:, c:c + 1], pt2[:], axis=AX.X, op=OP.add)
                    return mx, sm

                def emit_stats_d(p, st, mx, sm):
                    # em = exp(-scale*mx); d = 1 + sm*em.  Emitted after the
                    # first mm2 groups so no scan sigmoid is ready (d missing)
                    # while the early PSUM evacuations queue on ACT.
                    nc.scalar.activation(st["em"][:], mx[:], AF.Exp, scale=-SCALE)
                    tmp = stat_pool.tile([128, MT], F32, name=f"dt{p}",
                                         tag="dtmp")
                    nc.vector.tensor_tensor(out=tmp[:], in0=sm[:],
                                            in1=st["em"][:], op=OP.mult)
                    nc.vector.tensor_scalar_add(st["d"][:], tmp[:], 1.0)

                def emit_mm2(p, etiles, groups, otiles, evacs, st=None):
                    """out = e @ V, PSUM evacuated unscaled so the ACT queue
                    never waits on the reciprocal scan.  When st is given
                    (the pair's last groups, scan already done) the evac is
                    fused with the row scale on DVE, shortening the tail."""
                    for do, c in groups:
                            ps = ps2_pool.tile([128, 512], F32,
                                               name=f"o{p}_{do}_{c}", tag="ps2")
                            for nt in range(NT):
                                nc.tensor.matmul(
                                    ps[:],
                                    lhsT=etiles[nt][:, c * 128:(c + 1) * 128],
                                    rhs=vt[:, nt, do * 512:(do + 1) * 512],
                                    start=(nt == 0), stop=(nt == NT - 1),
                                )
                            ot = out_pool.tile([128, 512], F32,
                                               name=f"ot{p}_{do}_{c}", tag="ot")
                            if st is None:
                                ev = nc.scalar.activation(ot[:], ps[:], AF.Copy)
                                otiles[(do, c)] = ot
                                evacs.append(ev)
                            else:
                                nc.vector.tensor_scalar_mul(
                                    ot[:], ps[:], st["scale"][:, c:c + 1])
                                otiles[(do, c)] = None  # already scaled
                                m0 = p * PW + c * 128
                                nc.sync.dma_start(
                                    out=out_d[m0:m0 + 128,
                                              do * 512:(do + 1) * 512],
                                    in_=ot[:])

                def emit_scan(st, evacs):
                    """16-step sigmoid long-division on d: [128, 4] batched."""
                    d_t = st["d"]
                    r0 = stat_pool.tile([128, MT], F32, name="r0", tag="r0")
                    r1 = stat_pool.tile([128, MT], F32, name="r1", tag="r1")
                    q0 = stat_pool.tile([128, MT], F32, name="q0", tag="q0")
                    q1 = stat_pool.tile([128, MT], F32, name="q1", tag="q1")
                    z = stat_pool.tile([128, MT], F32, name="z", tag="z")
                    sg = stat_pool.tile([128, MT], F32, name="sg", tag="sg")
                    t = stat_pool.tile([128, MT], F32, name="t", tag="t")
                    nc.vector.memset(r0[:], 1.0)
                    nc.vector.memset(q0[:], 0.0)
                    r, qa = r0, q0
                    for i in range(BITS):
                        rn = r1 if r is r0 else r0
                        qn = q1 if qa is q0 else q0
                        nc.vector.scalar_tensor_tensor(      # z = 2r - d
                            out=z[:], in0=r[:], scalar=2.0, in1=d_t[:],
                            op0=OP.mult, op1=OP.subtract)
                        sig = nc.scalar.activation(          # step = sig(100 z)
                            sg[:], z[:], AF.Sigmoid, scale=SIG_SCALE)
                        if i >= 4:
                            # order-only hint: keep the slow sigmoid chain
                            # behind the PSUM evacuations in the ACT FIFO so
                            # mm2's psum recycling never waits on the scan
                            add_dep_helper(evacs[min(i - 4, len(evacs) - 1)].ins,
                                           sig.ins, True,
                                           "scan trails psum evacs")
                        nc.vector.tensor_tensor(             # t = d*step
                            out=t[:], in0=d_t[:], in1=sg[:], op=OP.mult)
                        nc.vector.scalar_tensor_tensor(      # r' = 2r - t
                            out=rn[:], in0=r[:], scalar=2.0, in1=t[:],
                            op0=OP.mult, op1=OP.subtract)
                        nc.vector.scalar_tensor_tensor(      # q' = w*step + q
                            out=qn[:], in0=sg[:], scalar=float(2.0 ** -(i + 1)),
                            in1=qa[:], op0=OP.mult, op1=OP.add)
                        r, qa = rn, qn
                    nc.vector.tensor_tensor(out=st["scale"][:], in0=st["em"][:],
                                            in1=qa[:], op=OP.mult)

                def emit_out(p, st, otiles):
                    """Apply the per-row scale and store."""
                    for do in range(DO):
                        for c in range(MT):
                            ot = otiles.get((do, c))
                            if ot is None:
                                continue
                            nc.vector.tensor_scalar_mul(
                                ot[:], ot[:], st["scale"][:, c:c + 1])
                            m0 = p * PW + c * 128
                            nc.sync.dma_start(
                                out=out_d[m0:m0 + 128, do * 512:(do + 1) * 512],
                                in_=ot[:])

                for p in range(NPAIR):
                    st = {
                        "em": stat_pool.tile([128, MT], F32, name=f"em{p}",
                                             tag="em"),
                        "d": stat_pool.tile([128, MT], F32, name=f"d{p}",
                                            tag="d"),
                        "scale": stat_pool.tile([128, MT], F32, name=f"sc{p}",
                                                tag="sc"),
                    }
                    etiles, macc, sacc = emit_mm1(p, st)
                    mx, sm = emit_stats(p, st, macc, sacc)
                    groups = [(do, c) for do in range(DO) for c in range(MT)]
                    otiles, evacs = {}, []
                    emit_mm2(p, etiles, groups[:2], otiles, evacs)
                    emit_stats_d(p, st, mx, sm)
                    emit_mm2(p, etiles, groups[2:6], otiles, evacs)
                    emit_scan(st, evacs)
                    emit_mm2(p, etiles, groups[6:], otiles, evacs, st=st)
                    emit_out(p, st, otiles)

    nc.compile()
    return nc


def _get_nc():
    if "nc" not in _CACHE:
        _CACHE["nc"] = _build()
    return _CACHE["nc"]


def kernel(query, keys, values):
    from concourse.bass_utils import run_bass_kernel_spmd

    query = np.ascontiguousarray(query, dtype=np.float32)
    keys = np.ascontiguousarray(keys, dtype=np.float32)
    values = np.ascontiguousarray(values, dtype=np.float32)

    nc = _get_nc()
    kT = np.ascontiguousarray(keys.T)
    in_maps = []
    for i in range(NCORES):
        qT = np.ascontiguousarray(query[i * M:(i + 1) * M].T)
        in_maps.append({"qT": qT, "kT": kT, "v": values})
    res = run_bass_kernel_spmd(nc, in_maps, list(range(NCORES)))
    out = np.concatenate([res.results[i]["out"] for i in range(NCORES)], axis=0)
    return np.ascontiguousarray(out, dtype=np.float32)

